# revision 31
# baseline (speedup 1.0000x reference)
"""AttentionConv Trainium2 kernel (8 NeuronCores, data-parallel over batch).

Reference math (per batch b, channel o, position (h,w), 7x7 window d=(di,dj)):
    q = wq @ x, k = wk @ x, v = wv @ x   (1x1 convs, channel matmuls)
    logits_d = q * k_d + rel             (k_d = zero-padded k shifted by d)
    out = sum_d softmax_d(logits) * v_d
`rel` is constant along the softmax axis, so it cancels. With zero padding,
out-of-bounds taps contribute exp(0)=1 to the denominator and 0 to the
numerator, which the zero-padded k/v slabs reproduce exactly.

v3 schedule (HW-measured: DVE bf16 tensor_tensor ~0.53ns/elem in 2x mode,
+3% for odd-offset windows; ACT 1/1.2GHz/elem; PE warm MM cadence 216ns at
N=512 with per-MM LDWEIGHTS overlapped by the reorder window):
  7 dj-groups; per group mega t-muls t7[7,32,64] = bcast(q) * kwin4D
  (stride-0 q + overlapping [72,72,1] window APs via AP stride surgery),
  split-in-2 14336-elem exps, mega u-muls, then 56 identity matmuls
  accumulating den/num in PSUM (per-MM LDWEIGHTS hides in the reorder
  window). DVE paces at ~15.4us/group; exp (12.5) and PE (12.1) hide
  under it. Odd-dj groups run first on the base slabs while ACT slack
  builds 1-shifted twin slabs that keep even-dj groups 4B-aligned.
  Tail: DVE reciprocal_approx_fast (saves the ACT ln/exp table + passes),
  bf16 output DMA. x ships from the host in bf16 (no on-chip casts).
Partition layout: p = g*64 + o for H-halves g in {0,1}; free dim = (32,64).
GPSIMD only does head border memsets (its SBUF port contends with DVE).
"""

import numpy as np
import ml_dtypes

import concourse.bass as bass
import concourse.tile as tile
from concourse import bacc, mybir
from concourse.bass_utils import run_bass_kernel_spmd

N_CORES = 8
B, C, H, W, O = 8, 64, 64, 64, 64
KS, PAD = 7, 3
HW = H * W                      # 4096
HG = H // 2                     # 32 rows per partition group
NHALF = HG * W                  # 2048 free elements per partition
RSLAB = HG + KS - 1             # 38 padded slab rows
LPAD = 4                        # left col pad
CSLAB = W + LPAD + PAD + 1      # 72 cols (even row stride)
SLAB = RSLAB * CSLAB            # 2736

F32 = mybir.dt.float32
BF16 = mybir.dt.bfloat16
_NPBF16 = ml_dtypes.bfloat16

CFG = {
    "tbufs": 2, "ebufs": 2, "ubufs": 2,
    "t_split": 2,                # t-mul mega-ops per group
    "u_split": 2,                # u-mul mega-ops per group (earlier PE start)
    "u_split_last": 4,           # finer on the last group (shorter PE drain)
    "exp_split": 2,              # ACTIVATEs per group
    # odd dj first: those windows are 4B-aligned in the base slabs, so the
    # 1-shifted twins (for even dj) can be built on ACT slack meanwhile
    "dj_order": (1, 3, 5, 0, 2, 4, 6),
}


def build_program():
    nc = bacc.Bacc("TRN2", target_bir_lowering=False, debug=False,
                   num_devices=N_CORES)

    x_d = nc.dram_tensor("x", [C, HW], BF16, kind="ExternalInput").ap()
    wqT_d = nc.dram_tensor("wqT", [C, O], BF16, kind="ExternalInput").ap()
    wkvT_d = nc.dram_tensor("wkvT", [C, 2 * O], BF16, kind="ExternalInput").ap()
    ident_d = nc.dram_tensor("ident", [128, 128], BF16, kind="ExternalInput").ap()
    out_d = nc.dram_tensor("out", [O, H, W], BF16, kind="ExternalOutput").ap()

    with tile.TileContext(nc) as tc:
        _build(tc, x_d, wqT_d, wkvT_d, ident_d, out_d)

    nc.compile()
    return nc


def _win(slab_flat, c0, di0, ndi):
    """[128, ndi, HG, W] overlapping window view of a [128, SLAB] slab:
    dims (di, r, c) with steps (CSLAB, CSLAB, 1), rows di0.., column c0."""
    t3 = slab_flat.rearrange("p (r c) -> p r c", r=RSLAB)
    v = t3[:, di0:di0 + HG, c0:c0 + W].unsqueeze(1).broadcast_to(
        [128, ndi, HG, W])
    w = v.copy()
    a = w.ap
    a[1] = [CSLAB, ndi]
    w.ap = a
    return w


def _build(tc, x_d, wqT_d, wkvT_d, ident_d, out_d):
    nc = tc.nc
    from contextlib import ExitStack

    with ExitStack() as ctx:
        konst = ctx.enter_context(tc.tile_pool(name="konst", bufs=1))
        big = ctx.enter_context(tc.tile_pool(name="big", bufs=1))

        # --- inputs to SBUF ---
        head = ExitStack()
        head_pool = head.enter_context(tc.tile_pool(name="head", bufs=1))
        # DMA order on the sync queue: wkT first (tiny), then x, then the
        # rest — the scalar queue takes ~11us to deliver, so everything the
        # projections need rides sync. ident is only needed at the first
        # den-matmul (~35us in).
        wkvT_sb = konst.tile([C, 2 * O], BF16, name="wkvT_sb")
        nc.sync.dma_start(wkvT_sb[:], wkvT_d[:])
        xb = head_pool.tile([C, HW], BF16, name="xb")
        for qtr in range(4):
            sl = slice(qtr * (HW // 4), (qtr + 1) * (HW // 4))
            nc.sync.dma_start(xb[:, sl], x_d[:, sl])
        wqT_sb = konst.tile([C, O], BF16, name="wqT_sb")
        nc.sync.dma_start(wqT_sb[:], wqT_d[:])
        ident_sb = konst.tile([128, 128], BF16, name="ident_sb")
        nc.sync.dma_start(ident_sb[:], ident_d[:])

        # --- padded k/v slabs (zeroed borders) + q ---
        q_sb = big.tile([128, HG, W], BF16, name="q_sb")
        kp = big.tile([128, SLAB], BF16, name="kp")
        vp = big.tile([128, SLAB], BF16, name="vp")
        kps = big.tile([128, SLAB], BF16, name="kps")
        vps = big.tile([128, SLAB], BF16, name="vps")
        kp3 = kp.rearrange("p (r c) -> p r c", r=RSLAB)
        vp3 = vp.rearrange("p (r c) -> p r c", r=RSLAB)
        # GPSIMD memsets run first (no deps) — off DVE's port before the loop
        for t3 in (kp3, vp3):
            nc.gpsimd.memset(t3[:, 0:PAD, :], 0.0)
            nc.gpsimd.memset(t3[:, RSLAB - PAD:RSLAB, :], 0.0)
            nc.gpsimd.memset(t3[:, PAD:RSLAB - PAD, 0:LPAD], 0.0)
            nc.gpsimd.memset(t3[:, PAD:RSLAB - PAD, LPAD + W:CSLAB], 0.0)

        proj_ctx = ExitStack()
        psum = proj_ctx.enter_context(
            tc.tile_pool(name="psum", bufs=4, space="PSUM"))


        # group 0 slab rows r hold image rows r-3 (valid r in [3,38));
        # group 1 slab rows r hold image rows r+29 (valid r in [0,35)).
        chunk_rows = [(0, 8), (8, 8), (16, 8), (24, 8), (32, 3)]

        # Stacked projection: one M=128 matmul computes k (out partitions
        # 0:64) AND v (64:128) per chunk per h-group — halves the cold-PE
        # matmul chain in the head. Same-partition evac halves ride ACT;
        # cross-partition halves ride DVE (quadrant-pair moves).
        for ci, (r0, nr) in enumerate(chunk_rows):
            n = nr * W
            psA = psum.tile([128, 512], F32, tag="proj", name=f"kvA{ci}")
            psB = psum.tile([128, 512], F32, tag="proj", name=f"kvB{ci}")
            nc.tensor.matmul(psA[:, :n], wkvT_sb[:],
                             xb[:, r0 * W:(r0 + nr) * W],
                             start=True, stop=True)
            nc.tensor.matmul(psB[:, :n], wkvT_sb[:],
                             xb[:, (29 + r0) * W:(29 + r0 + nr) * W],
                             start=True, stop=True)
            srcA = psA[:, :n].rearrange("p (a b) -> p a b", a=nr)
            srcB = psB[:, :n].rearrange("p (a b) -> p a b", a=nr)
            nc.scalar.copy(kp3[0:64, 3 + r0:3 + r0 + nr, LPAD:LPAD + W],
                           srcA[0:64])
            nc.vector.tensor_copy(vp3[0:64, 3 + r0:3 + r0 + nr, LPAD:LPAD + W],
                                  srcA[64:128])
            nc.vector.tensor_copy(kp3[64:128, r0:r0 + nr, LPAD:LPAD + W],
                                  srcB[0:64])
            nc.scalar.copy(vp3[64:128, r0:r0 + nr, LPAD:LPAD + W],
                           srcB[64:128])

        for cchunk in range(4):
            ps = psum.tile([128, 512], F32, tag="proj", name=f"q_ps{cchunk}")
            for g in (0, 1):
                rhs = xb[:, g * NHALF + cchunk * 512: g * NHALF + (cchunk + 1) * 512]
                nc.tensor.matmul(ps[g * 64:(g + 1) * 64, :], wqT_sb[:], rhs,
                                 start=True, stop=True)
            nc.vector.tensor_copy(
                q_sb[:, cchunk * 8:(cchunk + 1) * 8, :],
                ps[:].rearrange("p (a b) -> p a b", a=8))

        # 1-element-shifted twins keep even-dj groups 4B-aligned (+3% DVE
        # otherwise). Built on ACT slack while the odd-dj groups run first.
        nc.scalar.copy(kps[:, 0:SLAB - 1], kp[:, 1:SLAB])
        nc.scalar.copy(vps[:, 0:SLAB - 1], vp[:, 1:SLAB])

        proj_ctx.close()
        head.close()

        # --- main loop: 7 dj-groups x 7 di ---
        acc = ctx.enter_context(tc.tile_pool(name="acc", bufs=1, space="PSUM"))
        den_ps = acc.tile([128, NHALF], F32, name="den_ps")
        num_ps = acc.tile([128, NHALF], F32, name="num_ps")

        loop_ctx = ExitStack()
        tpool = loop_ctx.enter_context(tc.tile_pool(name="tpool", bufs=CFG["tbufs"]))
        epool = loop_ctx.enter_context(tc.tile_pool(name="epool", bufs=CFG["ebufs"]))
        upool = loop_ctx.enter_context(tc.tile_pool(name="upool", bufs=CFG["ubufs"]))

        qb = q_sb[:].unsqueeze(1).broadcast_to([128, KS, HG, W])

        def _sel(base, twin, c0):
            return (base, c0) if c0 % 2 == 0 else (twin, c0 - 1)

        n_off = KS * KS
        for gi, dj in enumerate(CFG["dj_order"]):
            c0 = dj + LPAD - PAD
            kb, kc = _sel(kp, kps, c0)
            vb, vc = _sel(vp, vps, c0)
            t7 = tpool.tile([128, KS, HG, W], BF16, tag="t", name=f"t7_{gi}")
            tsp = CFG["t_split"]
            tb = [round(KS * si / tsp) for si in range(tsp + 1)]
            for si in range(tsp):
                lo, hi = tb[si], tb[si + 1]
                nc.vector.tensor_mul(t7[:, lo:hi], qb[:, lo:hi],
                                     _win(kb, kc, lo, hi - lo))
            e7 = epool.tile([128, KS, HG, W], BF16, tag="e", name=f"e7_{gi}")
            nsp = CFG["exp_split"]
            t7f = t7.rearrange("p a b c -> p (a b c)")
            e7f = e7.rearrange("p a b c -> p (a b c)")
            bounds = [round(KS * si / nsp) * NHALF for si in range(nsp + 1)]
            for si in range(nsp):
                nc.scalar.activation(e7f[:, bounds[si]:bounds[si + 1]],
                                     t7f[:, bounds[si]:bounds[si + 1]],
                                     mybir.ActivationFunctionType.Exp)
            u7 = upool.tile([128, KS, HG, W], BF16, tag="u", name=f"u7_{gi}")
            usp = CFG["u_split_last"] if gi == KS - 1 else CFG["u_split"]
            ub = [round(KS * si / usp) for si in range(usp + 1)]
            for si in range(usp):
                lo, hi = ub[si], ub[si + 1]
                nc.vector.tensor_mul(u7[:, lo:hi], e7[:, lo:hi],
                                     _win(vb, vc, lo, hi - lo))
            for di in range(KS):
                d_global = gi * KS + di
                first = d_global == 0
                last = d_global == n_off - 1
                # last tap: all den matmuls first so the tail's reciprocal
                # can start while the num matmuls drain
                order = ([("d", cc) for cc in range(4)] +
                         [("n", cc) for cc in range(4)]) if last else \
                        [(w, cc) for cc in range(4) for w in ("d", "n")]
                for w, cc in order:
                    if w == "d":
                        nc.tensor.matmul(
                            den_ps[:, cc * 512:(cc + 1) * 512], ident_sb[:],
                            e7[:, di, cc * 8:(cc + 1) * 8, :],
                            start=first, stop=last, skip_group_check=True)
                    else:
                        nc.tensor.matmul(
                            num_ps[:, cc * 512:(cc + 1) * 512], ident_sb[:],
                            u7[:, di, cc * 8:(cc + 1) * 8, :],
                            start=first, stop=last, skip_group_check=True)

        loop_ctx.close()

        # --- divide and store (DVE approx reciprocal; den>0, well-scaled) ---
        tail_pool = ctx.enter_context(tc.tile_pool(name="tail", bufs=1))
        den_r = tail_pool.tile([128, NHALF], F32, name="den_r")
        out_sb = tail_pool.tile([128, NHALF], BF16, name="out_sb")
        out3 = out_sb.rearrange("p (a b) -> p a b", a=HG)
        for cc in range(4):
            sl = slice(cc * 512, (cc + 1) * 512)
            nc.vector.reciprocal_approx_fast(den_r[:, sl], den_ps[:, sl])
            nc.vector.tensor_mul(out_sb[:, sl], num_ps[:, sl], den_r[:, sl])
            rsl = slice(cc * 8, (cc + 1) * 8)
            nc.sync.dma_start(out_d[:, rsl, :], out3[0:64, rsl, :])
            nc.sync.dma_start(out_d[:, HG + cc * 8:HG + (cc + 1) * 8, :],
                              out3[64:128, rsl, :])


_NC_CACHE = None


def _get_nc():
    global _NC_CACHE
    if _NC_CACHE is None:
        _NC_CACHE = build_program()
    return _NC_CACHE


def prepare_in_maps(x, wq, wk, wv):
    x = np.ascontiguousarray(np.asarray(x, np.float32).astype(_NPBF16))
    wqT = np.ascontiguousarray(np.asarray(wq, np.float32).T.astype(_NPBF16))
    wkvT = np.ascontiguousarray(np.concatenate(
        [np.asarray(wk, np.float32).T, np.asarray(wv, np.float32).T],
        axis=1).astype(_NPBF16))
    ident = np.eye(128, dtype=_NPBF16)
    return [
        {"x": x[i].reshape(C, HW), "wqT": wqT, "wkvT": wkvT, "ident": ident}
        for i in range(x.shape[0])
    ]


def run(in_maps, **kw):
    nc = _get_nc()
    return run_bass_kernel_spmd(nc, in_maps, list(range(N_CORES)), **kw)


def kernel(x, wq, wk, wv, rel_w=None, rel_h=None, kernel_size=7, padding=3,
           **_ignored):
    # rel_w/rel_h are constant along the softmax axis, so they cancel.
    assert int(kernel_size) == KS and int(padding) == PAD
    res = run(prepare_in_maps(x, wq, wk, wv))
    out = np.stack([res.results[i]["out"] for i in range(N_CORES)], axis=0)
    return out.astype(np.float32)


if __name__ == "__main__":
    rng = np.random.default_rng(0)
    x = rng.standard_normal((B, C, H, W), dtype=np.float32)
    wq = (rng.standard_normal((O, C)) * 0.1).astype(np.float32)
    wk = (rng.standard_normal((O, C)) * 0.1).astype(np.float32)
    wv = (rng.standard_normal((O, C)) * 0.1).astype(np.float32)
    out = kernel(x, wq, wk, wv)
    print("out", out.shape, out.dtype, float(np.abs(out).max()))


# revision 32
# speedup vs baseline: 1.1812x; 1.1812x over previous
"""AttentionConv Trainium2 kernel (8 NeuronCores, data-parallel over batch).

Reference math (per batch b, channel o, position (h,w), 7x7 window d=(di,dj)):
    q = wq @ x, k = wk @ x, v = wv @ x   (1x1 convs, channel matmuls)
    logits_d = q * k_d + rel             (k_d = zero-padded k shifted by d)
    out = sum_d softmax_d(logits) * v_d
`rel` is constant along the softmax axis, so it cancels. With zero padding,
out-of-bounds taps contribute exp(0)=1 to the denominator and 0 to the
numerator, which the zero-padded k/v slabs reproduce exactly.

v3 schedule (HW-measured: DVE bf16 tensor_tensor ~0.53ns/elem in 2x mode,
+3% for odd-offset windows; ACT 1/1.2GHz/elem; PE warm MM cadence 216ns at
N=512 with per-MM LDWEIGHTS overlapped by the reorder window):
  7 dj-groups; per group ONE mega t-mul t7[7,32,64] = bcast(q) * kwin4D
  (stride-0 q + overlapping [72,72,1] window AP), ONE 14336-elem exp, ONE
  mega u-mul, then 56 identity matmuls accumulating den/num in PSUM.
  DVE paces at ~15.3us/group; exp (12.2) and PE (12.1) hide under it.
  Tail: DVE reciprocal_approx_fast (saves the ACT ln/exp table + passes).
Partition layout: p = g*64 + o for H-halves g in {0,1}; free dim = (32,64).
GPSIMD only does head border memsets (its SBUF port contends with DVE).
"""

import numpy as np
import ml_dtypes

import concourse.bass as bass
import concourse.tile as tile
from concourse import bacc, mybir
from concourse.bass_utils import run_bass_kernel_spmd

N_CORES = 8
B, C, H, W, O = 8, 64, 64, 64, 64
KS, PAD = 7, 3
HW = H * W                      # 4096
HG = H // 2                     # 32 rows per partition group
NHALF = HG * W                  # 2048 free elements per partition
RSLAB = HG + KS - 1             # 38 padded slab rows
LPAD = 4                        # left col pad
CSLAB = W + LPAD + PAD + 1      # 72 cols (even row stride)
SLAB = RSLAB * CSLAB            # 2736

F32 = mybir.dt.float32
BF16 = mybir.dt.bfloat16
_NPBF16 = ml_dtypes.bfloat16

CFG = {
    "tbufs": 2, "ebufs": 2, "ubufs": 2,
    "t_split": 2,                # t-mul mega-ops per group
    "u_split": 2,                # u-mul mega-ops per group (earlier PE start)
    "exp_split": 2,              # ACTIVATEs per group
    # odd dj first: those windows are 4B-aligned in the base slabs, so the
    # 1-shifted twins (for even dj) can be built on ACT slack meanwhile
    "dj_order": (1, 3, 5, 0, 2, 4, 6),
}


def build_program():
    nc = bacc.Bacc("TRN2", target_bir_lowering=False, debug=False,
                   num_devices=N_CORES)

    x_d = nc.dram_tensor("x", [C, HW], BF16, kind="ExternalInput").ap()
    wqT_d = nc.dram_tensor("wqT", [C, O], BF16, kind="ExternalInput").ap()
    wkT_d = nc.dram_tensor("wkT", [C, O], BF16, kind="ExternalInput").ap()
    wvT_d = nc.dram_tensor("wvT", [C, O], BF16, kind="ExternalInput").ap()
    ident_d = nc.dram_tensor("ident", [128, 128], BF16, kind="ExternalInput").ap()
    out_d = nc.dram_tensor("out", [O, H, W], BF16, kind="ExternalOutput").ap()

    with tile.TileContext(nc) as tc:
        _build(tc, x_d, wqT_d, wkT_d, wvT_d, ident_d, out_d)

    nc.compile()
    return nc


def _win(slab_flat, c0, di0, ndi):
    """[128, ndi, HG, W] overlapping window view of a [128, SLAB] slab:
    dims (di, r, c) with steps (CSLAB, CSLAB, 1), rows di0.., column c0."""
    t3 = slab_flat.rearrange("p (r c) -> p r c", r=RSLAB)
    v = t3[:, di0:di0 + HG, c0:c0 + W].unsqueeze(1).broadcast_to(
        [128, ndi, HG, W])
    w = v.copy()
    a = w.ap
    a[1] = [CSLAB, ndi]
    w.ap = a
    return w


def _build(tc, x_d, wqT_d, wkT_d, wvT_d, ident_d, out_d):
    nc = tc.nc
    from contextlib import ExitStack

    with ExitStack() as ctx:
        konst = ctx.enter_context(tc.tile_pool(name="konst", bufs=1))
        big = ctx.enter_context(tc.tile_pool(name="big", bufs=1))

        # --- inputs to SBUF ---
        head = ExitStack()
        head_pool = head.enter_context(tc.tile_pool(name="head", bufs=1))
        # DMA order on the sync queue: wkT first (tiny), then x, then the
        # rest — the scalar queue takes ~11us to deliver, so everything the
        # projections need rides sync. ident is only needed at the first
        # den-matmul (~35us in).
        wkT_sb = konst.tile([C, O], BF16, name="wkT_sb")
        nc.sync.dma_start(wkT_sb[:], wkT_d[:])
        xb = head_pool.tile([C, HW], BF16, name="xb")
        for qtr in range(4):
            sl = slice(qtr * (HW // 4), (qtr + 1) * (HW // 4))
            nc.sync.dma_start(xb[:, sl], x_d[:, sl])
        wqT_sb = konst.tile([C, O], BF16, name="wqT_sb")
        nc.sync.dma_start(wqT_sb[:], wqT_d[:])
        ident_sb = konst.tile([128, 128], BF16, name="ident_sb")
        nc.sync.dma_start(ident_sb[:], ident_d[:])
        wvT_sb = konst.tile([C, O], BF16, name="wvT_sb")
        nc.scalar.dma_start(wvT_sb[:], wvT_d[:])

        # --- padded k/v slabs (zeroed borders) + q ---
        q_sb = big.tile([128, HG, W], BF16, name="q_sb")
        kp = big.tile([128, SLAB], BF16, name="kp")
        vp = big.tile([128, SLAB], BF16, name="vp")
        kps = big.tile([128, SLAB], BF16, name="kps")
        vps = big.tile([128, SLAB], BF16, name="vps")
        kp3 = kp.rearrange("p (r c) -> p r c", r=RSLAB)
        vp3 = vp.rearrange("p (r c) -> p r c", r=RSLAB)
        # GPSIMD memsets run first (no deps) — off DVE's port before the loop
        for t3 in (kp3, vp3):
            nc.gpsimd.memset(t3[:, 0:PAD, :], 0.0)
            nc.gpsimd.memset(t3[:, RSLAB - PAD:RSLAB, :], 0.0)
            nc.gpsimd.memset(t3[:, PAD:RSLAB - PAD, 0:LPAD], 0.0)
            nc.gpsimd.memset(t3[:, PAD:RSLAB - PAD, LPAD + W:CSLAB], 0.0)

        proj_ctx = ExitStack()
        psum = proj_ctx.enter_context(
            tc.tile_pool(name="psum", bufs=4, space="PSUM"))


        # group 0 slab rows r hold image rows r-3 (valid r in [3,38));
        # group 1 slab rows r hold image rows r+29 (valid r in [0,35)).
        chunk_rows = [(0, 8), (8, 8), (16, 8), (24, 8), (32, 3)]

        def project_kv(wT_sb, dst3, name, evac_even, evac_odd):
            for ci, (r0, nr) in enumerate(chunk_rows):
                n = nr * W
                ps = psum.tile([128, 512], F32, tag="proj", name=f"{name}_ps{ci}")
                nc.tensor.matmul(ps[0:64, :n], wT_sb[:],
                                 xb[:, r0 * W:(r0 + nr) * W],
                                 start=True, stop=True)
                nc.tensor.matmul(ps[64:128, :n], wT_sb[:],
                                 xb[:, (29 + r0) * W:(29 + r0 + nr) * W],
                                 start=True, stop=True)
                src = ps[:, :n].rearrange("p (a b) -> p a b", a=nr)
                evac = evac_even if ci % 2 == 0 else evac_odd
                evac(dst3[0:64, 3 + r0:3 + r0 + nr, LPAD:LPAD + W], src[0:64])
                evac(dst3[64:128, r0:r0 + nr, LPAD:LPAD + W], src[64:128])

        # k first (the first t-mul needs it); evacs split DVE/ACT
        project_kv(wkT_sb, kp3, "k", nc.vector.tensor_copy, nc.scalar.copy)

        for cchunk in range(4):
            ps = psum.tile([128, 512], F32, tag="proj", name=f"q_ps{cchunk}")
            for g in (0, 1):
                rhs = xb[:, g * NHALF + cchunk * 512: g * NHALF + (cchunk + 1) * 512]
                nc.tensor.matmul(ps[g * 64:(g + 1) * 64, :], wqT_sb[:], rhs,
                                 start=True, stop=True)
            nc.vector.tensor_copy(
                q_sb[:, cchunk * 8:(cchunk + 1) * 8, :],
                ps[:].rearrange("p (a b) -> p a b", a=8))

        project_kv(wvT_sb, vp3, "v", nc.scalar.copy, nc.scalar.copy)
        # 1-element-shifted twins keep even-dj groups 4B-aligned (+3% DVE
        # otherwise). Built on ACT slack while the odd-dj groups run first.
        nc.scalar.copy(kps[:, 0:SLAB - 1], kp[:, 1:SLAB])
        nc.scalar.copy(vps[:, 0:SLAB - 1], vp[:, 1:SLAB])

        proj_ctx.close()
        head.close()

        # --- main loop: 7 dj-groups x 7 di ---
        acc = ctx.enter_context(tc.tile_pool(name="acc", bufs=1, space="PSUM"))
        den_ps = acc.tile([128, NHALF], F32, name="den_ps")
        num_ps = acc.tile([128, NHALF], F32, name="num_ps")

        loop_ctx = ExitStack()
        tpool = loop_ctx.enter_context(tc.tile_pool(name="tpool", bufs=CFG["tbufs"]))
        epool = loop_ctx.enter_context(tc.tile_pool(name="epool", bufs=CFG["ebufs"]))
        upool = loop_ctx.enter_context(tc.tile_pool(name="upool", bufs=CFG["ubufs"]))

        qb = q_sb[:].unsqueeze(1).broadcast_to([128, KS, HG, W])

        def _sel(base, twin, c0):
            return (base, c0) if c0 % 2 == 0 else (twin, c0 - 1)

        n_off = KS * KS
        for gi, dj in enumerate(CFG["dj_order"]):
            c0 = dj + LPAD - PAD
            kb, kc = _sel(kp, kps, c0)
            vb, vc = _sel(vp, vps, c0)
            t7 = tpool.tile([128, KS, HG, W], BF16, tag="t", name=f"t7_{gi}")
            tsp = CFG["t_split"]
            tb = [round(KS * si / tsp) for si in range(tsp + 1)]
            for si in range(tsp):
                lo, hi = tb[si], tb[si + 1]
                nc.vector.tensor_mul(t7[:, lo:hi], qb[:, lo:hi],
                                     _win(kb, kc, lo, hi - lo))
            e7 = epool.tile([128, KS, HG, W], BF16, tag="e", name=f"e7_{gi}")
            nsp = CFG["exp_split"]
            t7f = t7.rearrange("p a b c -> p (a b c)")
            e7f = e7.rearrange("p a b c -> p (a b c)")
            bounds = [round(KS * si / nsp) * NHALF for si in range(nsp + 1)]
            for si in range(nsp):
                nc.scalar.activation(e7f[:, bounds[si]:bounds[si + 1]],
                                     t7f[:, bounds[si]:bounds[si + 1]],
                                     mybir.ActivationFunctionType.Exp)
            u7 = upool.tile([128, KS, HG, W], BF16, tag="u", name=f"u7_{gi}")
            usp = CFG["u_split"]
            ub = [round(KS * si / usp) for si in range(usp + 1)]
            for si in range(usp):
                lo, hi = ub[si], ub[si + 1]
                nc.vector.tensor_mul(u7[:, lo:hi], e7[:, lo:hi],
                                     _win(vb, vc, lo, hi - lo))
            for di in range(KS):
                d_global = gi * KS + di
                first = d_global == 0
                last = d_global == n_off - 1
                # last tap: all den matmuls first so the tail's reciprocal
                # can start while the num matmuls drain
                order = ([("d", cc) for cc in range(4)] +
                         [("n", cc) for cc in range(4)]) if last else \
                        [(w, cc) for cc in range(4) for w in ("d", "n")]
                for w, cc in order:
                    if w == "d":
                        nc.tensor.matmul(
                            den_ps[:, cc * 512:(cc + 1) * 512], ident_sb[:],
                            e7[:, di, cc * 8:(cc + 1) * 8, :],
                            start=first, stop=last, skip_group_check=True)
                    else:
                        nc.tensor.matmul(
                            num_ps[:, cc * 512:(cc + 1) * 512], ident_sb[:],
                            u7[:, di, cc * 8:(cc + 1) * 8, :],
                            start=first, stop=last, skip_group_check=True)

        loop_ctx.close()

        # --- divide and store (DVE approx reciprocal; den>0, well-scaled) ---
        tail_pool = ctx.enter_context(tc.tile_pool(name="tail", bufs=1))
        den_r = tail_pool.tile([128, NHALF], F32, name="den_r")
        out_sb = tail_pool.tile([128, NHALF], BF16, name="out_sb")
        out3 = out_sb.rearrange("p (a b) -> p a b", a=HG)
        for cc in range(4):
            sl = slice(cc * 512, (cc + 1) * 512)
            nc.vector.reciprocal_approx_fast(den_r[:, sl], den_ps[:, sl])
            nc.vector.tensor_mul(out_sb[:, sl], num_ps[:, sl], den_r[:, sl])
            rsl = slice(cc * 8, (cc + 1) * 8)
            nc.sync.dma_start(out_d[:, rsl, :], out3[0:64, rsl, :])
            nc.sync.dma_start(out_d[:, HG + cc * 8:HG + (cc + 1) * 8, :],
                              out3[64:128, rsl, :])


_NC_CACHE = None


def _get_nc():
    global _NC_CACHE
    if _NC_CACHE is None:
        _NC_CACHE = build_program()
    return _NC_CACHE


def prepare_in_maps(x, wq, wk, wv):
    x = np.ascontiguousarray(np.asarray(x, np.float32).astype(_NPBF16))
    wqT = np.ascontiguousarray(np.asarray(wq, np.float32).T.astype(_NPBF16))
    wkT = np.ascontiguousarray(np.asarray(wk, np.float32).T.astype(_NPBF16))
    wvT = np.ascontiguousarray(np.asarray(wv, np.float32).T.astype(_NPBF16))
    ident = np.eye(128, dtype=_NPBF16)
    return [
        {"x": x[i].reshape(C, HW), "wqT": wqT, "wkT": wkT, "wvT": wvT,
         "ident": ident}
        for i in range(x.shape[0])
    ]


def run(in_maps, **kw):
    nc = _get_nc()
    return run_bass_kernel_spmd(nc, in_maps, list(range(N_CORES)), **kw)


def kernel(x, wq, wk, wv, rel_w=None, rel_h=None, kernel_size=7, padding=3,
           **_ignored):
    # rel_w/rel_h are constant along the softmax axis, so they cancel.
    assert int(kernel_size) == KS and int(padding) == PAD
    res = run(prepare_in_maps(x, wq, wk, wv))
    out = np.stack([res.results[i]["out"] for i in range(N_CORES)], axis=0)
    return out.astype(np.float32)


if __name__ == "__main__":
    rng = np.random.default_rng(0)
    x = rng.standard_normal((B, C, H, W), dtype=np.float32)
    wq = (rng.standard_normal((O, C)) * 0.1).astype(np.float32)
    wk = (rng.standard_normal((O, C)) * 0.1).astype(np.float32)
    wv = (rng.standard_normal((O, C)) * 0.1).astype(np.float32)
    out = kernel(x, wq, wk, wv)
    print("out", out.shape, out.dtype, float(np.abs(out).max()))
